# revision 24
# baseline (speedup 1.0000x reference)
"""Trainium2 Bass kernel for nn_DisentangledHierarchicalEncoder.

Strategy (8 NeuronCores, SPMD, zero collectives):
  The whole pipeline is per-item: token (b,s) output = F(seq[b,s]) where F
  mixes the 4 modality embeddings of one item, so the device computes F once
  per *unique* item (~32k of 50k; ~4009/core) and the host replicates rows to
  tokens afterward.  Host shards the unique items across cores, pre-gathers
  their raw features in feature-major [feat, item] layout, and pre-normalizes
  content/text (the original model l2-normalizes those tables offline in
  __init__).  Each core runs a dense pipeline in 8 chunks of 512 items:
      content MLP (1024->1024->256->64), text MLP (768->768->256->64) in bf16,
      cf linear (64->64), id passthrough,
      fused l2norm+LayerNorm folded to xn = (e - mu) * rsqrt(ss*(1/64+eps)-mu^2),
      4x4 self-attention (G = wq.T @ wk / 8), mean pool, wv projection.
  Attention avoids PE broadcast matmuls: per-token stats land in
  quadrant-local PSUM rows (stat lhsT patterns) and DVE stream_shuffle
  (mask [r]*32) broadcasts them across partitions; softmax row ops stay in
  [16, C] lane-aligned layout.  MLP evac is a single Act relu per m-tile.
  PSUM banks are split pbig(3 MLP) / pattn(2 qg+fps) / psmall(3 stats) so
  attention never blocks MLP accumulation.
"""

import numpy as np

NUM_ITEM = 50000
B, S, D = 1024, 50, 64
DC, DT = 1024, 768
N_CORES = 8
C = 512                           # chunk width (items per chunk)
NCH = 8                           # chunks per core
T = C * NCH                       # 4096 padded unique items per core
KC, KT_ = DC // 128, DT // 128    # k-tiles: 8 content, 6 text
LN_EPS = 1e-5
MLP_BF16 = True                   # L1/L2 matmuls in bf16

_CACHE = {}

MASK0 = [0] * 32
MASK1 = [1] * 32


def _bf():
    if not MLP_BF16:
        return np.float32
    import ml_dtypes
    return ml_dtypes.bfloat16


def _build_consts(inputs):
    f32 = np.float32
    # stats lhsT pack [128, 4, 128]:
    #   slice 0 (rhs emb_ct):  col 32q+0 = 1/64 on half(q) -> st_mu local row 0
    #   slice 1 (rhs emb_cfid): col 32q+1 = 1/64 on half(q) -> st_mu local row 1
    #   slice 2 (rhs sq_ct):   col 32q+0 = 1 on half(q)    -> st_ss local row 0
    #   slice 3 (rhs sq_cfid): col 32q+1 = 1 on half(q)    -> st_ss local row 1
    # where half(q) = top 64 partitions for quadrants 0,1 else bottom 64.
    statp = np.zeros((128, 4, 128), f32)
    for q in range(4):
        half = slice(0, 64) if q < 2 else slice(64, 128)
        statp[half, 0, 32 * q + 0] = 1.0 / 64
        statp[half, 1, 32 * q + 1] = 1.0 / 64
        statp[half, 2, 32 * q + 0] = 1.0
        statp[half, 3, 32 * q + 1] = 1.0
    # score-reduce lhsT [128, 8, 16]; s16 row 4m+n.
    # slice 2m   (rhs pr1 = qg_m * xn_ct):   col 4m+0 top, col 4m+1 bottom
    # slice 2m+1 (rhs pr2 = qg_m * xn_cfid): col 4m+2 top, col 4m+3 bottom
    redp = np.zeros((128, 8, 16), f32)
    for m in range(4):
        redp[0:64, 2 * m, 4 * m + 0] = 1.0
        redp[64:128, 2 * m, 4 * m + 1] = 1.0
        redp[0:64, 2 * m + 1, 4 * m + 2] = 1.0
        redp[64:128, 2 * m + 1, 4 * m + 3] = 1.0
    # Z16 = rowsum over n within each m block: block-diagonal 4x4 ones
    zsum = np.zeros((16, 16), f32)
    for m in range(4):
        zsum[4 * m:4 * m + 4, 4 * m:4 * m + 4] = 1.0
    # a_bc [128, C]: quadrant q: local row 0 = a_{c|t}, local row 1 = a_{cf|id}
    # a_n = 0.25 * sum_m w16[4m+n]
    asum = np.zeros((16, 128), f32)
    for q in range(4):
        n0 = 0 if q < 2 else 1
        n1 = 2 if q < 2 else 3
        for m in range(4):
            asum[4 * m + n0, 32 * q + 0] = 0.25
            asum[4 * m + n1, 32 * q + 1] = 0.25
    # attention weights
    wq = np.asarray(inputs["wq"], dtype=f32)
    wk = np.asarray(inputs["wk"], dtype=f32)
    wv = np.asarray(inputs["wv"], dtype=f32)
    G = (wq.T @ wk) * (D ** -0.5)
    G2 = np.concatenate([G, G], axis=1)            # [64, 128]
    g2t = np.concatenate([G2, np.zeros((64, 128), f32)], axis=0)
    g2b = np.concatenate([np.zeros((64, 128), f32), G2], axis=0)
    wv2 = np.concatenate([wv.T, wv.T], axis=0)     # [128, 64]
    return dict(statp=statp, redp=redp, zsum=zsum, asum=asum,
                g2t=g2t, g2b=g2b, wv2=wv2)


def _build_nc(nch=NCH, n_cores=N_CORES, repeat=1):
    import concourse.bacc as bacc
    import concourse.tile as tile
    from concourse import mybir
    from contextlib import ExitStack

    T = C * nch
    FR = mybir.dt.float32r
    F32 = mybir.dt.float32
    BF = mybir.dt.bfloat16 if MLP_BF16 else FR
    AF = mybir.ActivationFunctionType

    nc = bacc.Bacc("TRN2", target_bir_lowering=False, debug=False,
                   num_devices=n_cores)

    din = {}
    def dt_in(name, shape, dt=FR):
        din[name] = nc.dram_tensor(name, list(shape), dt, kind="ExternalInput")
        return din[name]

    # per-chunk SBUF-ordered layout: one contiguous KC*C (8KB) descriptor per
    # partition per chunk-load instead of KC separate 1KB lines
    xc = dt_in("xc", [nch, 128, KC * C], BF)
    xt = dt_in("xt", [nch, 128, KT_ * C], BF)
    xcf = dt_in("xcf", [64, T])
    xid = dt_in("xid", [64, T])
    cw1 = dt_in("cw1", [DC, DC], BF)
    cw2 = dt_in("cw2", [DC, 256], BF)
    tw1 = dt_in("tw1", [DT, DT], BF)
    tw2 = dt_in("tw2", [DT, 256], BF)
    w3p = dt_in("w3p", [128, 4, 128])
    cfwp = dt_in("cfwp", [64, 128])
    statp = dt_in("statp", [128, 4, 128])
    redp = dt_in("redp", [128, 8, 16])
    zsum = dt_in("zsum", [16, 16])
    asum = dt_in("asum", [16, 128])
    g2t = dt_in("g2t", [128, 128])
    g2b = dt_in("g2b", [128, 128])
    wv2 = dt_in("wv2", [128, 64])
    b1c = dt_in("b1c", [KC, 128], F32)
    b2c = dt_in("b2c", [2, 128], F32)
    b1t = dt_in("b1t", [KT_, 128], F32)
    b2t = dt_in("b2t", [2, 128], F32)
    b3 = dt_in("b3", [128, 1], F32)
    bcf = dt_in("bcf", [64, 1], F32)
    out = nc.dram_tensor("out", [64, T], F32, kind="ExternalOutput")

    xc_r = xc.rearrange("j p (kt c) -> j p kt c", kt=KC)
    xt_r = xt.rearrange("j p (kt c) -> j p kt c", kt=KT_)

    from concourse.alu_op_type import AluOpType as ALU

    with nc.allow_low_precision("float32r tiles feed float32r matmuls by design"), \
            tile.TileContext(nc) as tc:
        with ExitStack() as ctx:
            wp = ctx.enter_context(tc.tile_pool(name="wp", bufs=1))
            xin = ctx.enter_context(tc.tile_pool(name="xin", bufs=2))
            h1p = ctx.enter_context(tc.tile_pool(name="h1p", bufs=2))
            h2p = ctx.enter_context(tc.tile_pool(name="h2p", bufs=1))
            sqp = ctx.enter_context(tc.tile_pool(name="sqp", bufs=2))
            embp = ctx.enter_context(tc.tile_pool(name="embp", bufs=2))
            xnp = ctx.enter_context(tc.tile_pool(name="xnp", bufs=2))
            bcp = ctx.enter_context(tc.tile_pool(name="bcp", bufs=1))
            abp = ctx.enter_context(tc.tile_pool(name="abp", bufs=1))
            prp = ctx.enter_context(tc.tile_pool(name="prp", bufs=6))
            smp = ctx.enter_context(tc.tile_pool(name="smp", bufs=2))
            outp = ctx.enter_context(tc.tile_pool(name="outp", bufs=2))
            pbig = ctx.enter_context(tc.tile_pool(name="pbig", bufs=3,
                                                  space="PSUM"))
            pattn = ctx.enter_context(tc.tile_pool(name="pattn", bufs=3,
                                                   space="PSUM"))
            psmall = ctx.enter_context(tc.tile_pool(name="psmall", bufs=2,
                                                    space="PSUM"))

            # ---- resident weights / consts ----
            cw1s = wp.tile([128, KC, DC], BF)
            nc.sync.dma_start(out=cw1s, in_=cw1.rearrange("(kt p) m -> p kt m", p=128))
            cw2s = wp.tile([128, KC, 256], BF)
            nc.sync.dma_start(out=cw2s, in_=cw2.rearrange("(kt p) m -> p kt m", p=128))
            tw1s = wp.tile([128, KT_, DT], BF)
            nc.sync.dma_start(out=tw1s, in_=tw1.rearrange("(kt p) m -> p kt m", p=128))
            tw2s = wp.tile([128, KT_, 256], BF)
            nc.sync.dma_start(out=tw2s, in_=tw2.rearrange("(kt p) m -> p kt m", p=128))
            w3ps = wp.tile([128, 4, 128], FR)
            nc.sync.dma_start(out=w3ps, in_=w3p[:, :, :])
            cfwps = wp.tile([64, 128], FR)
            nc.sync.dma_start(out=cfwps, in_=cfwp[:, :])
            statps = wp.tile([128, 4, 128], FR)
            nc.sync.dma_start(out=statps, in_=statp[:, :, :])
            redps = wp.tile([128, 8, 16], FR)
            nc.sync.dma_start(out=redps, in_=redp[:, :, :])
            zsums = wp.tile([16, 16], FR)
            nc.sync.dma_start(out=zsums, in_=zsum[:, :])
            asums = wp.tile([16, 128], FR)
            nc.sync.dma_start(out=asums, in_=asum[:, :])
            g2ts = wp.tile([128, 128], FR)
            nc.sync.dma_start(out=g2ts, in_=g2t[:, :])
            g2bs = wp.tile([128, 128], FR)
            nc.sync.dma_start(out=g2bs, in_=g2b[:, :])
            wv2s = wp.tile([128, 64], FR)
            nc.sync.dma_start(out=wv2s, in_=wv2[:, :])
            b1cs = wp.tile([128, KC], F32)
            nc.sync.dma_start(out=b1cs, in_=b1c.rearrange("m p -> p m"))
            b2cs = wp.tile([128, 2], F32)
            nc.sync.dma_start(out=b2cs, in_=b2c.rearrange("m p -> p m"))
            b1ts = wp.tile([128, KT_], F32)
            nc.sync.dma_start(out=b1ts, in_=b1t.rearrange("m p -> p m"))
            b2ts = wp.tile([128, 2], F32)
            nc.sync.dma_start(out=b2ts, in_=b2t.rearrange("m p -> p m"))
            b3s = wp.tile([128, 1], F32)
            nc.sync.dma_start(out=b3s, in_=b3[:, :])
            bcfs = wp.tile([64, 1], F32)
            nc.sync.dma_start(out=bcfs, in_=bcf[:, :])

            state = {}

            def make_units(j):
                """MLP work for chunk j, as a list of emit fns."""
                sl = slice(j * C, (j + 1) * C)
                ctx_j = {}

                def u_load():
                    xc_j = xin.tile([128, KC, C], BF, tag="xc")
                    nc.sync.dma_start(out=xc_j, in_=xc_r[j, :, :, :])
                    xt_j = xin.tile([128, KT_, C], BF, tag="xt")
                    nc.sync.dma_start(out=xt_j, in_=xt_r[j, :, :, :])
                    xcf_j = xin.tile([64, C], FR, tag="xcf")
                    nc.sync.dma_start(out=xcf_j, in_=xcf[:, sl])
                    emb_cfid = embp.tile([128, C], FR, tag="emb_cfid")
                    nc.sync.dma_start(out=emb_cfid[64:128, :], in_=xid[:, sl])
                    ctx_j.update(xc_j=xc_j, xt_j=xt_j, xcf_j=xcf_j,
                                 emb_cfid=emb_cfid)

                def l1_tile(xj_key, kt, w1s, b1sT, tag, m):
                    def emit():
                        xj = ctx_j[xj_key]
                        if m == 0:
                            h1 = h1p.tile([128, kt, C], BF, tag="h1" + tag)
                            ctx_j["h1" + tag] = h1
                        h1 = ctx_j["h1" + tag]
                        ps = pbig.tile([128, C], F32, tag="mm", name="ps")
                        for k in range(kt):
                            nc.tensor.matmul(
                                ps[:, :], w1s[:, k, 128 * m:128 * (m + 1)],
                                xj[:, k, :], start=(k == 0), stop=(k == kt - 1))
                        nc.scalar.activation(out=h1[:, m, :], in_=ps[:, :],
                                             func=AF.Relu,
                                             bias=b1sT[:, m:m + 1])
                    return emit

                def l2_tile(kt, w2s, b2sT, tag, m):
                    def emit():
                        h1 = ctx_j["h1" + tag]
                        if m == 0:
                            h2 = h2p.tile([128, 2, C], FR, tag="h2" + tag)
                            ctx_j["h2" + tag] = h2
                        h2 = ctx_j["h2" + tag]
                        ps = pbig.tile([128, C], F32, tag="mm")
                        for k in range(kt):
                            nc.tensor.matmul(
                                ps[:, :], w2s[:, k, 128 * m:128 * (m + 1)],
                                h1[:, k, :], start=(k == 0), stop=(k == kt - 1))
                        nc.scalar.activation(out=h2[:, m, :], in_=ps[:, :],
                                             func=AF.Relu,
                                             bias=b2sT[:, m:m + 1])
                    return emit

                def u_l3cf():
                    h2c, h2t = ctx_j["h2c"], ctx_j["h2t"]
                    ps3 = pbig.tile([128, C], F32, tag="mm")
                    nc.tensor.matmul(ps3[:, :], w3ps[:, 0, :], h2c[:, 0, :],
                                     start=True, stop=False)
                    nc.tensor.matmul(ps3[:, :], w3ps[:, 1, :], h2c[:, 1, :],
                                     start=False, stop=False)
                    nc.tensor.matmul(ps3[:, :], w3ps[:, 2, :], h2t[:, 0, :],
                                     start=False, stop=False)
                    nc.tensor.matmul(ps3[:, :], w3ps[:, 3, :], h2t[:, 1, :],
                                     start=False, stop=True)
                    emb_ct = embp.tile([128, C], FR, tag="emb_ct")
                    nc.scalar.activation(out=emb_ct, in_=ps3[:, :],
                                         func=AF.Identity, bias=b3s[:, :])
                    pcf = pbig.tile([128, C], F32, tag="mm")
                    nc.tensor.matmul(pcf[:, :], cfwps[:, :], ctx_j["xcf_j"],
                                     start=True, stop=True)
                    emb_cfid = ctx_j["emb_cfid"]
                    nc.scalar.activation(out=emb_cfid[0:64, :],
                                         in_=pcf[0:64, :],
                                         func=AF.Identity, bias=bcfs[:, :])
                    state[j] = (emb_ct, emb_cfid)

                units = [u_load]
                units += [l1_tile("xc_j", KC, cw1s, b1cs, "c", m)
                          for m in range(KC)]
                units += [l1_tile("xt_j", KT_, tw1s, b1ts, "t", m)
                          for m in range(KT_)]
                units += [l2_tile(KC, cw2s, b2cs, "c", m) for m in range(2)]
                units += [l2_tile(KT_, tw2s, b2ts, "t", m) for m in range(2)]
                units += [u_l3cf]
                return units

            def make_stages(j):
                """attention for chunk j (embs from state[j]), as emit fns."""
                sl = slice(j * C, (j + 1) * C)
                actx = {}

                def s_sq():
                    emb_ct, emb_cfid = state.pop(j)
                    actx["emb_ct"], actx["emb_cfid"] = emb_ct, emb_cfid
                    sq_ct = sqp.tile([128, C], FR, tag="sqct", name="sq_ct")
                    nc.gpsimd.tensor_mul(sq_ct, emb_ct, emb_ct)
                    sq_cfid = sqp.tile([128, C], FR, tag="sqcfid",
                                       name="sq_cfid")
                    nc.gpsimd.tensor_mul(sq_cfid, emb_cfid, emb_cfid)
                    actx["sq_ct"], actx["sq_cfid"] = sq_ct, sq_cfid

                def s_stats():
                    emb_ct, emb_cfid = actx["emb_ct"], actx["emb_cfid"]
                    sq_ct, sq_cfid = actx["sq_ct"], actx["sq_cfid"]
                    st_mu = psmall.tile([128, C], F32, tag="sm", name="st_mu")
                    nc.tensor.matmul(st_mu[:, :], statps[:, 0, :], emb_ct,
                                     start=True, stop=False)
                    nc.tensor.matmul(st_mu[:, :], statps[:, 1, :], emb_cfid,
                                     start=False, stop=True)
                    st_ss = psmall.tile([128, C], F32, tag="sm", name="st_ss")
                    nc.tensor.matmul(st_ss[:, :], statps[:, 2, :], sq_ct,
                                     start=True, stop=False)
                    nc.tensor.matmul(st_ss[:, :], statps[:, 3, :], sq_cfid,
                                     start=False, stop=True)
                    actx["st_mu"], actx["st_ss"] = st_mu, st_ss

                def s_ab():
                    # A = rsqrt(ss*(1/64+eps) - mu^2); junk lanes unread
                    st_mu, st_ss = actx["st_mu"], actx["st_ss"]
                    musq = abp.tile([128, C], FR, tag="musq", name="musq")
                    nc.scalar.activation(out=musq, in_=st_mu[:, :],
                                         func=AF.Square)
                    apre = abp.tile([128, C], FR, tag="apre", name="apre")
                    nc.vector.scalar_tensor_tensor(
                        apre, st_ss[:, :], 1.0 / 64 + LN_EPS, musq,
                        op0=ALU.mult, op1=ALU.subtract)
                    asq = abp.tile([128, C], FR, tag="asq", name="asq")
                    nc.scalar.activation(out=asq, in_=apre, func=AF.Sqrt)
                    A128 = abp.tile([128, C], F32, tag="A128", name="A128")
                    nc.vector.reciprocal(A128, asq)
                    actx["A128"] = A128

                def s_bc():
                    st_mu, A128 = actx["st_mu"], actx["A128"]
                    mub_ct = bcp.tile([128, C], F32, tag="mub_ct")
                    nc.vector.stream_shuffle(mub_ct, st_mu[:, :], MASK0)
                    mub_cfid = bcp.tile([128, C], F32, tag="mub_cfid")
                    nc.vector.stream_shuffle(mub_cfid, st_mu[:, :], MASK1)
                    ab_ct = bcp.tile([128, C], F32, tag="ab_ct")
                    nc.vector.stream_shuffle(ab_ct, A128, MASK0)
                    ab_cfid = bcp.tile([128, C], F32, tag="ab_cfid")
                    nc.vector.stream_shuffle(ab_cfid, A128, MASK1)
                    actx.update(mub_ct=mub_ct, mub_cfid=mub_cfid,
                                ab_ct=ab_ct, ab_cfid=ab_cfid)

                def s_xn():
                    xc_ct = prp.tile([128, C], FR, tag="pr", name="xc_ct")
                    nc.gpsimd.tensor_sub(xc_ct, actx["emb_ct"],
                                         actx["mub_ct"])
                    xn_ct = xnp.tile([128, C], FR, tag="xnct")
                    nc.vector.tensor_mul(xn_ct, xc_ct, actx["ab_ct"])
                    xc_cfid = prp.tile([128, C], FR, tag="pr", name="xc_cfid")
                    nc.vector.tensor_sub(xc_cfid, actx["emb_cfid"],
                                         actx["mub_cfid"])
                    xn_cfid = xnp.tile([128, C], FR, tag="xncfid")
                    nc.gpsimd.tensor_mul(xn_cfid, xc_cfid, actx["ab_cfid"])
                    actx["xnct"], actx["xncfid"] = xn_ct, xn_cfid

                def s_qg_a(m):
                    def emit():
                        gsel = g2ts if m % 2 == 0 else g2bs
                        xn_src = actx["xnct"] if m < 2 else actx["xncfid"]
                        qg = pattn.tile([128, C], F32, tag="qg", name="qg")
                        nc.tensor.matmul(qg[:, :], gsel[:, :], xn_src,
                                         start=True, stop=True)
                        pr1 = prp.tile([128, C], FR, tag="pr", name="pr1")
                        nc.vector.tensor_mul(pr1, qg[:, :], actx["xnct"])
                        pr2 = prp.tile([128, C], FR, tag="pr", name="pr2")
                        nc.vector.tensor_mul(pr2, qg[:, :], actx["xncfid"])
                        actx[f"pr{m}"] = (pr1, pr2)
                    return emit

                def s_qg_b(m):
                    def emit():
                        if m == 0:
                            actx["s16"] = psmall.tile([16, C], F32, tag="sm",
                                                      name="s16")
                        pr1, pr2 = actx.pop(f"pr{m}")
                        s16 = actx["s16"]
                        nc.tensor.matmul(s16[:, :], redps[:, 2 * m, :], pr1,
                                         start=(m == 0), stop=False)
                        nc.tensor.matmul(s16[:, :], redps[:, 2 * m + 1, :],
                                         pr2, start=False, stop=(m == 3))
                    return emit

                def s_exp():
                    e16 = smp.tile([16, C], FR, tag="e16")
                    nc.scalar.activation(out=e16, in_=actx["s16"][:, :],
                                         func=AF.Exp)
                    actx["e16"] = e16

                def s_zrz():
                    Z16 = psmall.tile([16, C], F32, tag="sm", name="Z16")
                    nc.tensor.matmul(Z16[:, :], zsums[:, :], actx["e16"],
                                     start=True, stop=True)
                    rz16 = smp.tile([16, C], FR, tag="rz16")
                    nc.vector.reciprocal(rz16, Z16[:, :])
                    actx["rz16"] = rz16

                def s_w16():
                    w16 = smp.tile([16, C], FR, tag="w16")
                    nc.gpsimd.tensor_mul(w16, actx["e16"], actx["rz16"])
                    actx["w16"] = w16

                def s_abc():
                    a_bc = psmall.tile([128, C], F32, tag="sm", name="a_bc")
                    nc.tensor.matmul(a_bc[:, :], asums[:, :], actx["w16"],
                                     start=True, stop=True)
                    ab_ct = bcp.tile([128, C], F32, tag="avct")
                    nc.vector.stream_shuffle(ab_ct, a_bc[:, :], MASK0)
                    ab_cfid = bcp.tile([128, C], F32, tag="avcfid")
                    nc.vector.stream_shuffle(ab_cfid, a_bc[:, :], MASK1)
                    actx["avct"], actx["avcfid"] = ab_ct, ab_cfid

                def s_zp():
                    zp_ct = prp.tile([128, C], FR, tag="pr", name="zp_ct")
                    nc.vector.tensor_mul(zp_ct, actx["xnct"], actx["avct"])
                    zp_cfid = prp.tile([128, C], FR, tag="pr", name="zp_cfid")
                    nc.gpsimd.tensor_mul(zp_cfid, actx["xncfid"],
                                         actx["avcfid"])
                    actx["zps"] = (zp_ct, zp_cfid)

                def s_val():
                    zp_ct, zp_cfid = actx.pop("zps")
                    fps = pattn.tile([64, C], F32, tag="qg", name="fps")
                    nc.tensor.matmul(fps[:, :], wv2s[:, :], zp_ct,
                                     start=True, stop=False)
                    nc.tensor.matmul(fps[:, :], wv2s[:, :], zp_cfid,
                                     start=False, stop=True)
                    out_sb = outp.tile([64, C], F32, tag="osb")
                    nc.vector.tensor_copy(out_sb, fps[:, :])
                    nc.sync.dma_start(out=out[:, sl], in_=out_sb)

                return [s_sq, s_stats, s_ab, s_bc, s_xn,
                        s_qg_a(0), s_qg_b(0), s_qg_a(1), s_qg_b(1),
                        s_qg_a(2), s_qg_b(2), s_qg_a(3), s_qg_b(3),
                        s_exp, s_zrz, s_w16, s_abc, s_zp, s_val]

            import os
            skip_attn = bool(int(os.environ.get("K_SKIP_ATTN", "0")))
            skip_mlp = bool(int(os.environ.get("K_SKIP_MLP", "0")))

            def emit_all():
                for it in range(nch + 1):
                    units = make_units(it) if it < nch else []
                    stages = (make_stages(it - 1) if it >= 1 else [])
                    if skip_attn:
                        stages = []
                        if it >= 1:
                            state.pop(it - 1, None)
                    if skip_mlp:
                        units = units[:1]  # loads only
                        stages = []
                    # weave: distribute stages evenly among units
                    n_u, n_s = len(units), len(stages)
                    si = 0
                    for ui, u in enumerate(units):
                        u()
                        want = ((ui + 1) * n_s) // max(n_u, 1)
                        while si < want:
                            stages[si]()
                            si += 1
                    while si < n_s:
                        stages[si]()
                        si += 1

            if repeat == 1:
                emit_all()
            else:
                with tc.For_i(0, repeat, 1):
                    emit_all()

    nc.finalize()
    return nc


def _get_nc():
    if "nc" not in _CACHE:
        _CACHE["nc"] = _build_nc()
    return _CACHE["nc"]


def _dedup_layout(seq):
    """The whole pipeline is per-item (a token's 4 modalities all come from
    item seq[b,s] and attention mixes only those), so the device computes one
    fused embedding per *unique* item; the host expands to tokens afterward.
    Returns (per-core padded item lists, item -> global column map)."""
    u = np.unique(seq.reshape(-1))
    n_per = -(-len(u) // N_CORES)
    item2pos = np.zeros(NUM_ITEM, np.int64)
    core_idx = []
    for c in range(N_CORES):
        uc = u[c * n_per:(c + 1) * n_per]
        item2pos[uc] = c * T + np.arange(len(uc))
        core_idx.append(np.concatenate([uc, np.zeros(T - len(uc), np.int64)]))
    return core_idx, item2pos


def _host_prep(inputs, T=T):
    f32 = np.float32
    seq = np.asarray(inputs["seq_modify"])
    seq = np.where(seq == NUM_ITEM, 0, seq).astype(np.int64)  # [1024, 50]
    cf_full = np.asarray(inputs["content_feature"], dtype=f32)
    tf_full = np.asarray(inputs["text_feature"], dtype=f32)
    # pre-normalize the static tables (reference l2-normalizes in __init__)
    cf_full = cf_full / np.maximum(
        np.sqrt((cf_full * cf_full).sum(axis=1, keepdims=True)), 1e-12)
    tf_full = tf_full / np.maximum(
        np.sqrt((tf_full * tf_full).sum(axis=1, keepdims=True)), 1e-12)
    cff_full = np.asarray(inputs["cf_feature"], dtype=f32)
    ide_full = np.asarray(inputs["item_embeddings"], dtype=f32)

    c_w3 = np.asarray(inputs["c_w3"], dtype=f32)   # [64, 256]
    t_w3 = np.asarray(inputs["t_w3"], dtype=f32)
    cw3T = np.ascontiguousarray(c_w3.T)            # [256, 64]
    tw3T = np.ascontiguousarray(t_w3.T)
    w3p = np.zeros((128, 4, 128), f32)
    w3p[:, 0, 0:64] = cw3T[0:128]
    w3p[:, 1, 0:64] = cw3T[128:256]
    w3p[:, 2, 64:128] = tw3T[0:128]
    w3p[:, 3, 64:128] = tw3T[128:256]
    cf_w = np.asarray(inputs["cf_w"], dtype=f32)
    cfwp = np.zeros((64, 128), f32)
    cfwp[:, 0:64] = cf_w.T
    b3 = np.concatenate([np.asarray(inputs["c_b3"], dtype=f32),
                         np.asarray(inputs["t_b3"], dtype=f32)])[:, None]

    shared = dict(
        cw1=np.ascontiguousarray(np.asarray(inputs["c_w1"], dtype=f32).T
                                 .astype(_bf())),
        cw2=np.ascontiguousarray(np.asarray(inputs["c_w2"], dtype=f32).T
                                 .astype(_bf())),
        tw1=np.ascontiguousarray(np.asarray(inputs["t_w1"], dtype=f32).T
                                 .astype(_bf())),
        tw2=np.ascontiguousarray(np.asarray(inputs["t_w2"], dtype=f32).T
                                 .astype(_bf())),
        w3p=w3p, cfwp=cfwp,
        b1c=np.asarray(inputs["c_b1"], dtype=f32).reshape(KC, 128),
        b2c=np.asarray(inputs["c_b2"], dtype=f32).reshape(2, 128),
        b1t=np.asarray(inputs["t_b1"], dtype=f32).reshape(KT_, 128),
        b2t=np.asarray(inputs["t_b2"], dtype=f32).reshape(2, 128),
        b3=b3,
        bcf=np.asarray(inputs["cf_b"], dtype=f32)[:, None],
        **_build_consts(inputs),
    )

    def _chunked(gathered, kt):
        # [T, kt*128] -> [NCH, 128, kt*C] in per-chunk SBUF order
        a = gathered.reshape(NCH, C, kt, 128).transpose(0, 3, 2, 1)
        return np.ascontiguousarray(a.reshape(NCH, 128, kt * C))

    core_idx, _ = _dedup_layout(seq)
    in_maps = []
    for c in range(N_CORES):
        idx = core_idx[c]
        m = dict(shared)
        m["xc"] = _chunked(cf_full[idx].astype(_bf()), KC)
        m["xt"] = _chunked(tf_full[idx].astype(_bf()), KT_)
        m["xcf"] = np.ascontiguousarray(cff_full[idx].T)
        m["xid"] = np.ascontiguousarray(ide_full[idx].T)
        in_maps.append(m)
    return in_maps


def _get_runner(nc=None, key="runner"):
    """Cached jitted shard_map runner over 8 cores (mirrors
    bass2jax.run_bass_via_pjrt but reuses one jit so repeat calls skip
    retracing)."""
    if key in _CACHE:
        return _CACHE[key]
    import jax
    from jax.sharding import Mesh, PartitionSpec
    try:
        from jax.experimental.shard_map import shard_map
    except ImportError:
        from jax.shard_map import shard_map
    from concourse import bass2jax, mybir

    if nc is None:
        nc = _get_nc()
    bass2jax.install_neuronx_cc_hook()
    partition_name = (nc.partition_id_tensor.name
                      if nc.partition_id_tensor else None)
    in_names, out_names, out_avals, zero_shapes = [], [], [], []
    for alloc in nc.m.functions[0].allocations:
        if not isinstance(alloc, mybir.MemoryLocationSet):
            continue
        name = alloc.memorylocations[0].name
        if alloc.kind == "ExternalInput":
            if name != partition_name:
                in_names.append(name)
        elif alloc.kind == "ExternalOutput":
            out_names.append(name)
            shape = tuple(alloc.tensor_shape)
            dtype = mybir.dt.np(alloc.dtype)
            out_avals.append(jax.core.ShapedArray(shape, dtype))
            zero_shapes.append((shape, dtype))
    n_params = len(in_names)
    full_in_names = list(in_names) + list(out_names)
    if partition_name is not None:
        full_in_names.append(partition_name)

    def _body(*args):
        operands = list(args)
        if partition_name is not None:
            operands.append(bass2jax.partition_id_tensor())
        outs = bass2jax._bass_exec_p.bind(
            *operands,
            out_avals=tuple(out_avals),
            in_names=tuple(full_in_names),
            out_names=tuple(out_names),
            lowering_input_output_aliases=(),
            sim_require_finite=True,
            sim_require_nnan=True,
            nc=nc,
        )
        return tuple(outs)

    devices = jax.devices()[:N_CORES]
    mesh = Mesh(np.asarray(devices), ("core",))
    n_outs = len(out_names)
    in_specs = (PartitionSpec("core"),) * (n_params + n_outs)
    out_specs = (PartitionSpec("core"),) * n_outs
    sharded = jax.jit(
        shard_map(_body, mesh=mesh, in_specs=in_specs, out_specs=out_specs,
                  check_rep=False),
        keep_unused=True,
    )
    runner = (sharded, in_names, out_names, zero_shapes, mesh)
    _CACHE[key] = runner
    return runner


def _run_device(in_maps):
    sharded, in_names, out_names, zero_shapes, _ = _get_runner()
    concat_in = [
        np.concatenate([np.asarray(in_maps[c][n]) for c in range(N_CORES)],
                       axis=0)
        for n in in_names
    ]
    concat_zeros = [np.zeros((N_CORES * s[0], *s[1:]), d)
                    for (s, d) in zero_shapes]
    out_arrs = sharded(*concat_in, *concat_zeros)
    return np.asarray(out_arrs[out_names.index("out")])


def kernel(**inputs):
    in_maps = _host_prep(inputs)
    out_cat = _run_device(in_maps)          # [8*64, T]
    seq = np.asarray(inputs["seq_modify"])
    seq = np.where(seq == NUM_ITEM, 0, seq).astype(np.int64)
    _, item2pos = _dedup_layout(seq)
    table = np.concatenate(
        [out_cat[c * 64:(c + 1) * 64, :] for c in range(N_CORES)], axis=1)
    full = table[:, item2pos[seq]]          # [64, B, S]
    return np.ascontiguousarray(np.moveaxis(full, 0, -1), dtype=np.float32)
